# revision 3
# baseline (speedup 1.0000x reference)
"""Trainium2 Bass kernel for CombinedRotaryEmbedding.

Computation (per token vector of h_dim=64, per head):
  1. xr = xh @ M          with M = (compose of 32 Givens rotations) @ rotation_matrix
  2. RoPE: x1 = xr[0::2], x2 = xr[1::2]
     out = concat([x1*cos - x2*sin, x1*sin + x2*cos])  (cos/sin of pos * inv_freq)

Device strategy (8 cores, shard S=4096 into 8 x 512 positions, all batches per core):
  - Host precomputes M' = M with columns permuted so xr' = (x1 | x2), and
    M2 = blockdiag(M', M') so one 128x128 matmul covers a pair of heads.
  - Host precomputes cos/sin tables replicated per head -> per-core inputs.
  - Per 128-token tile [128, 1024]:
      DMA in (natural layout) -> PE transpose (8x 128x128 blocks)
      -> ScalarE copy PSUM->SBUF -> PE matmul vs M2 (8 blocks, natural out)
      -> DVE: 4 multiplies vs cos/sin tables (PSUM source)
      -> GPSIMD: sub/add to final SBUF tile -> DMA out.
"""

import sys

sys.path.insert(0, "/opt/trn_rl_repo")

import numpy as np

B, S, N_STATE, N_HEAD = 8, 4096, 1024, 16
H_DIM = N_STATE // N_HEAD  # 64
NUM_ROT = H_DIM // 2  # 32
BASE = 10000.0
N_CORES = 8
S_SHARD = S // N_CORES  # 512
ROWS = B * S_SHARD  # 4096 token rows per core
N_TILES = ROWS // 128  # 32 tiles per core; tile t: b = t//4, s-chunk = t%4

_MODULE_CACHE = {}


def _build_module():
    if "nc" in _MODULE_CACHE:
        return _MODULE_CACHE["nc"]

    import concourse.mybir as mybir
    from concourse import bacc
    from concourse.tile import TileContext

    f32 = mybir.dt.float32
    nc = bacc.Bacc(name="rotary")

    x = nc.dram_tensor("x", [ROWS, N_STATE], f32, kind="ExternalInput")
    m2 = nc.dram_tensor("m2", [128, 128], f32, kind="ExternalInput")
    ident = nc.dram_tensor("ident", [128, 128], f32, kind="ExternalInput")
    # cos/sin tables: [128 partitions, (chunk=4, head=16, j=32)]
    crep = nc.dram_tensor("crep", [128, 2048], f32, kind="ExternalInput")
    srep = nc.dram_tensor("srep", [128, 2048], f32, kind="ExternalInput")
    out = nc.dram_tensor("out", [ROWS, N_STATE], f32, kind="ExternalOutput")

    with TileContext(nc) as tc:
        with (
            tc.tile_pool(name="const", bufs=1) as cpool,
            tc.tile_pool(name="io", bufs=3) as iopool,
            tc.tile_pool(name="work", bufs=2) as wpool,
            tc.tile_pool(name="ps_t", bufs=2, space="PSUM") as pstp,
            tc.tile_pool(name="ps_r", bufs=2, space="PSUM") as psrp,
        ):
            m2_sb = cpool.tile([128, 128], f32, name="m2_sb")
            nc.sync.dma_start(out=m2_sb, in_=m2[:, :])
            ident_sb = cpool.tile([128, 128], f32, name="ident_sb")
            nc.sync.dma_start(out=ident_sb, in_=ident[:, :])
            crep_sb = cpool.tile([128, 4, 16, 32], f32, name="crep_sb")
            nc.sync.dma_start(
                out=crep_sb, in_=crep.rearrange("p (c h j) -> p c h j", c=4, h=16)
            )
            srep_sb = cpool.tile([128, 4, 16, 32], f32, name="srep_sb")
            nc.sync.dma_start(
                out=srep_sb, in_=srep.rearrange("p (c h j) -> p c h j", c=4, h=16)
            )

            for t in range(N_TILES):
                chunk = t % 4
                x_nat = iopool.tile([128, N_STATE], f32, name="x_nat", tag="xin")
                nc.sync.dma_start(out=x_nat, in_=x[t * 128 : (t + 1) * 128, :])

                ps_t = pstp.tile([128, N_STATE], f32, name="ps_t", tag="pst")
                for k in range(8):
                    nc.tensor.transpose(
                        ps_t[:, k * 128 : (k + 1) * 128],
                        x_nat[:, k * 128 : (k + 1) * 128],
                        ident_sb,
                    )

                xT = wpool.tile([128, N_STATE], f32, name="xT", tag="xT")
                nc.scalar.copy(out=xT, in_=ps_t)

                ps_r = psrp.tile([128, N_STATE], f32, name="ps_r", tag="psr")
                for k in range(8):
                    nc.tensor.matmul(
                        ps_r[:, k * 128 : (k + 1) * 128],
                        lhsT=xT[:, k * 128 : (k + 1) * 128],
                        rhs=m2_sb,
                        start=True,
                        stop=True,
                    )

                # RoPE elementwise. ps_r free layout = (h=16, half=2, j=32)
                xr = ps_r.rearrange("p (h t j) -> p h t j", h=16, t=2)
                x1 = xr[:, :, 0, :]
                x2 = xr[:, :, 1, :]
                c_t = crep_sb[:, chunk]
                s_t = srep_sb[:, chunk]

                t1 = wpool.tile([128, 16, 32], f32, name="t1", tag="t1")
                t2 = wpool.tile([128, 16, 32], f32, name="t2", tag="t2")
                t3 = wpool.tile([128, 16, 32], f32, name="t3", tag="t3")
                t4 = wpool.tile([128, 16, 32], f32, name="t4", tag="t4")
                nc.vector.tensor_mul(out=t1, in0=x1, in1=c_t)
                nc.vector.tensor_mul(out=t2, in0=x2, in1=s_t)
                nc.vector.tensor_mul(out=t3, in0=x1, in1=s_t)
                nc.vector.tensor_mul(out=t4, in0=x2, in1=c_t)

                o_sb = iopool.tile([128, N_STATE], f32, name="o_sb", tag="out")
                o_r = o_sb.rearrange("p (h t j) -> p h t j", h=16, t=2)
                nc.gpsimd.tensor_sub(out=o_r[:, :, 0, :], in0=t1, in1=t2)
                nc.gpsimd.tensor_add(out=o_r[:, :, 1, :], in0=t3, in1=t4)

                nc.sync.dma_start(out=out[t * 128 : (t + 1) * 128, :], in_=o_sb)

    nc.finalize()
    _MODULE_CACHE["nc"] = nc
    return nc


def _host_params(thetas, rotation_pairs, theta_scale, rotation_matrix, inv_freq):
    """Replicates the reference's composition of Givens rotations in f32."""
    idx = rotation_pairs.astype(np.int32)
    th = (thetas.astype(np.float32) * np.float32(theta_scale[0])).astype(np.float32)
    R = np.eye(H_DIM, dtype=np.float32)
    for k in range(NUM_ROT):
        i, j = int(idx[k, 0]), int(idx[k, 1])
        c = np.float32(np.cos(th[k]))
        sn = np.float32(np.sin(th[k]))
        G = np.eye(H_DIM, dtype=np.float32)
        G[i, i] = c
        G[i, j] = -sn
        G[j, i] = sn
        G[j, j] = c
        R = (R @ G).astype(np.float32)
    M = (R @ rotation_matrix.astype(np.float32)).astype(np.float32)
    # column permutation: first 32 cols = even outputs (x1), last 32 = odd (x2)
    perm = np.concatenate([np.arange(0, H_DIM, 2), np.arange(1, H_DIM, 2)])
    Mp = M[:, perm]
    m2 = np.zeros((128, 128), dtype=np.float32)
    m2[:64, :64] = Mp
    m2[64:, 64:] = Mp

    pos = np.arange(S, dtype=np.float32)
    sinu = pos[:, None] * inv_freq.astype(np.float32)[None, :]  # [4096, 32]
    cos = np.cos(sinu).astype(np.float32)
    sin = np.sin(sinu).astype(np.float32)
    return m2, cos, sin


def _rep_table(tab_core):
    """[512, 32] -> [128, 2048] laid out as [p, (chunk=4, h=16, j=32)]."""
    t = tab_core.reshape(4, 128, 32).transpose(1, 0, 2)  # [128, 4, 32]
    t = np.broadcast_to(t[:, :, None, :], (128, 4, 16, 32))
    return np.ascontiguousarray(t.reshape(128, 2048), dtype=np.float32)


def kernel(**inputs):
    from concourse.bass_utils import run_bass_kernel_spmd

    x = np.ascontiguousarray(np.asarray(inputs["x"], dtype=np.float32))
    m2, cos, sin = _host_params(
        np.asarray(inputs["thetas"], np.float32),
        np.asarray(inputs["rotation_pairs"], np.float32),
        np.asarray(inputs["theta_scale"], np.float32),
        np.asarray(inputs["rotation_matrix"], np.float32),
        np.asarray(inputs["inv_freq"], np.float32),
    )
    ident = np.eye(128, dtype=np.float32)

    nc = _build_module()
    in_maps = []
    for c in range(N_CORES):
        xs = np.ascontiguousarray(
            x[:, c * S_SHARD : (c + 1) * S_SHARD, :].reshape(ROWS, N_STATE)
        )
        in_maps.append(
            {
                "x": xs,
                "m2": m2,
                "ident": ident,
                "crep": _rep_table(cos[c * S_SHARD : (c + 1) * S_SHARD]),
                "srep": _rep_table(sin[c * S_SHARD : (c + 1) * S_SHARD]),
            }
        )

    res = run_bass_kernel_spmd(nc, in_maps, core_ids=list(range(N_CORES)))
    out = np.empty((B, S, N_STATE), dtype=np.float32)
    for c in range(N_CORES):
        out[:, c * S_SHARD : (c + 1) * S_SHARD, :] = res.results[c]["out"].reshape(
            B, S_SHARD, N_STATE
        )
    return out


# revision 5
# speedup vs baseline: 263.8492x; 263.8492x over previous
"""Trainium2 Bass kernel for CombinedRotaryEmbedding.

Computation (per token vector of h_dim=64, per head):
  1. xr = xh @ M          with M = (compose of 32 Givens rotations) @ rotation_matrix
  2. RoPE: x1 = xr[0::2], x2 = xr[1::2]
     out = concat([x1*cos - x2*sin, x1*sin + x2*cos])  (cos/sin of pos * inv_freq)

Device strategy (8 cores, shard S=4096 into 8 x 512 positions, all batches per core):
  - Host precomputes M' = M with columns permuted so xr' = (x1 | x2), and
    M2 = blockdiag(M', M') so one 128x128 matmul covers a pair of heads.
  - Host precomputes cos/sin tables replicated per head -> per-core inputs.
  - Per 128-token tile [128, 1024]:
      DMA in (natural layout) -> PE transpose (8x 128x128 blocks)
      -> ScalarE copy PSUM->SBUF -> PE matmul vs M2 (8 blocks, natural out)
      -> DVE: 4 multiplies vs cos/sin tables (PSUM source)
      -> GPSIMD: sub/add to final SBUF tile -> DMA out.
"""

import sys

sys.path.insert(0, "/opt/trn_rl_repo")

import numpy as np

B, S, N_STATE, N_HEAD = 8, 4096, 1024, 16
H_DIM = N_STATE // N_HEAD  # 64
NUM_ROT = H_DIM // 2  # 32
BASE = 10000.0
N_CORES = 8
S_SHARD = S // N_CORES  # 512
ROWS = B * S_SHARD  # 4096 token rows per core
N_TILES = ROWS // 128  # 32 tiles per core; tile t: b = t//4, s-chunk = t%4

_MODULE_CACHE = {}


def _build_module(n_tiles=N_TILES):
    key = ("nc", n_tiles)
    if key in _MODULE_CACHE:
        return _MODULE_CACHE[key]

    import concourse.mybir as mybir
    from concourse import bacc
    from concourse.tile import TileContext

    f32 = mybir.dt.float32
    nc = bacc.Bacc(name="rotary")

    x = nc.dram_tensor("x", [ROWS, N_STATE], f32, kind="ExternalInput")
    m2 = nc.dram_tensor("m2", [128, 128], f32, kind="ExternalInput")
    ident = nc.dram_tensor("ident", [128, 128], f32, kind="ExternalInput")
    # cos/sin tables: [128 partitions, (chunk=4, head=16, j=32)]
    crep = nc.dram_tensor("crep", [128, 2048], f32, kind="ExternalInput")
    srep = nc.dram_tensor("srep", [128, 2048], f32, kind="ExternalInput")
    out = nc.dram_tensor("out", [ROWS, N_STATE], f32, kind="ExternalOutput")

    with TileContext(nc) as tc:
        with (
            tc.tile_pool(name="const", bufs=1) as cpool,
            tc.tile_pool(name="io", bufs=3) as iopool,
            tc.tile_pool(name="work", bufs=2) as wpool,
            tc.tile_pool(name="ps_t", bufs=2, space="PSUM") as pstp,
            tc.tile_pool(name="ps_r", bufs=2, space="PSUM") as psrp,
        ):
            m2_sb = cpool.tile([128, 128], f32, name="m2_sb")
            nc.sync.dma_start(out=m2_sb, in_=m2[:, :])
            ident_sb = cpool.tile([128, 128], f32, name="ident_sb")
            nc.sync.dma_start(out=ident_sb, in_=ident[:, :])
            crep_sb = cpool.tile([128, 4, 16, 32], f32, name="crep_sb")
            nc.sync.dma_start(
                out=crep_sb, in_=crep.rearrange("p (c h j) -> p c h j", c=4, h=16)
            )
            srep_sb = cpool.tile([128, 4, 16, 32], f32, name="srep_sb")
            nc.sync.dma_start(
                out=srep_sb, in_=srep.rearrange("p (c h j) -> p c h j", c=4, h=16)
            )

            for t in range(n_tiles):
                chunk = t % 4
                x_nat = iopool.tile([128, N_STATE], f32, name="x_nat", tag="xin")
                nc.sync.dma_start(out=x_nat, in_=x[t * 128 : (t + 1) * 128, :])

                ps_t = pstp.tile([128, N_STATE], f32, name="ps_t", tag="pst")
                for k in range(8):
                    nc.tensor.transpose(
                        ps_t[:, k * 128 : (k + 1) * 128],
                        x_nat[:, k * 128 : (k + 1) * 128],
                        ident_sb,
                    )

                xT = wpool.tile([128, N_STATE], f32, name="xT", tag="xT")
                nc.scalar.copy(out=xT, in_=ps_t)

                ps_r = psrp.tile([128, N_STATE], f32, name="ps_r", tag="psr")
                for k in range(8):
                    nc.tensor.matmul(
                        ps_r[:, k * 128 : (k + 1) * 128],
                        lhsT=xT[:, k * 128 : (k + 1) * 128],
                        rhs=m2_sb,
                        start=True,
                        stop=True,
                    )

                # RoPE elementwise. ps_r free layout = (h=16, half=2, j=32)
                xr = ps_r.rearrange("p (h t j) -> p h t j", h=16, t=2)
                x1 = xr[:, :, 0, :]
                x2 = xr[:, :, 1, :]
                c_t = crep_sb[:, chunk]
                s_t = srep_sb[:, chunk]

                t1 = wpool.tile([128, 16, 32], f32, name="t1", tag="t1")
                t2 = wpool.tile([128, 16, 32], f32, name="t2", tag="t2")
                t3 = wpool.tile([128, 16, 32], f32, name="t3", tag="t3")
                t4 = wpool.tile([128, 16, 32], f32, name="t4", tag="t4")
                nc.vector.tensor_mul(out=t1, in0=x1, in1=c_t)
                nc.vector.tensor_mul(out=t2, in0=x2, in1=s_t)
                nc.vector.tensor_mul(out=t3, in0=x1, in1=s_t)
                nc.vector.tensor_mul(out=t4, in0=x2, in1=c_t)

                o_sb = iopool.tile([128, N_STATE], f32, name="o_sb", tag="out")
                o_r = o_sb.rearrange("p (h t j) -> p h t j", h=16, t=2)
                nc.gpsimd.tensor_sub(out=o_r[:, :, 0, :], in0=t1, in1=t2)
                nc.gpsimd.tensor_add(out=o_r[:, :, 1, :], in0=t3, in1=t4)

                nc.sync.dma_start(out=out[t * 128 : (t + 1) * 128, :], in_=o_sb)

    nc.finalize()
    _MODULE_CACHE[key] = nc
    return nc


def _host_params(thetas, rotation_pairs, theta_scale, rotation_matrix, inv_freq):
    """Replicates the reference's composition of Givens rotations in f32."""
    idx = rotation_pairs.astype(np.int32)
    th = (thetas.astype(np.float32) * np.float32(theta_scale[0])).astype(np.float32)
    R = np.eye(H_DIM, dtype=np.float32)
    for k in range(NUM_ROT):
        i, j = int(idx[k, 0]), int(idx[k, 1])
        c = np.float32(np.cos(th[k]))
        sn = np.float32(np.sin(th[k]))
        G = np.eye(H_DIM, dtype=np.float32)
        G[i, i] = c
        G[i, j] = -sn
        G[j, i] = sn
        G[j, j] = c
        R = (R @ G).astype(np.float32)
    M = (R @ rotation_matrix.astype(np.float32)).astype(np.float32)
    # column permutation: first 32 cols = even outputs (x1), last 32 = odd (x2)
    perm = np.concatenate([np.arange(0, H_DIM, 2), np.arange(1, H_DIM, 2)])
    Mp = M[:, perm]
    m2 = np.zeros((128, 128), dtype=np.float32)
    m2[:64, :64] = Mp
    m2[64:, 64:] = Mp

    pos = np.arange(S, dtype=np.float32)
    sinu = pos[:, None] * inv_freq.astype(np.float32)[None, :]  # [4096, 32]
    cos = np.cos(sinu).astype(np.float32)
    sin = np.sin(sinu).astype(np.float32)
    return m2, cos, sin


def _rep_table(tab_core):
    """[512, 32] -> [128, 2048] laid out as [p, (chunk=4, h=16, j=32)]."""
    t = tab_core.reshape(4, 128, 32).transpose(1, 0, 2)  # [128, 4, 32]
    t = np.broadcast_to(t[:, :, None, :], (128, 4, 16, 32))
    return np.ascontiguousarray(t.reshape(128, 2048), dtype=np.float32)


def kernel(**inputs):
    from concourse.bass_utils import run_bass_kernel_spmd

    x = np.ascontiguousarray(np.asarray(inputs["x"], dtype=np.float32))
    m2, cos, sin = _host_params(
        np.asarray(inputs["thetas"], np.float32),
        np.asarray(inputs["rotation_pairs"], np.float32),
        np.asarray(inputs["theta_scale"], np.float32),
        np.asarray(inputs["rotation_matrix"], np.float32),
        np.asarray(inputs["inv_freq"], np.float32),
    )
    ident = np.eye(128, dtype=np.float32)

    nc = _build_module()
    in_maps = []
    for c in range(N_CORES):
        xs = np.ascontiguousarray(
            x[:, c * S_SHARD : (c + 1) * S_SHARD, :].reshape(ROWS, N_STATE)
        )
        in_maps.append(
            {
                "x": xs,
                "m2": m2,
                "ident": ident,
                "crep": _rep_table(cos[c * S_SHARD : (c + 1) * S_SHARD]),
                "srep": _rep_table(sin[c * S_SHARD : (c + 1) * S_SHARD]),
            }
        )

    res = run_bass_kernel_spmd(nc, in_maps, core_ids=list(range(N_CORES)))
    out = np.empty((B, S, N_STATE), dtype=np.float32)
    for c in range(N_CORES):
        out[:, c * S_SHARD : (c + 1) * S_SHARD, :] = res.results[c]["out"].reshape(
            B, S_SHARD, N_STATE
        )
    return out
